# revision 9
# baseline (speedup 1.0000x reference)
"""GroupedQueryAttention Trainium2 kernel (8 NeuronCores).

Sharding: core i handles (batch b = i//4, KV group g = i%4): its 4 query
heads + 1 KV group, full sequence. Each core computes a partial output
(attn_heads @ Wo rows for its heads); host sums the 4 partials per batch.

Layout strategy (per core):
  - everything transposed: qT/kT [d, t] computed with W-stationary matmuls
  - RoPE: host permutes W rows to half-split layout; swap-half via a
    permutation matmul on PE; cos/sin tables applied on DVE
  - attention: scoresT [s, tq] = kT_tile^T @ qT (stationary kT tile),
    exp on ACT (no max subtraction -- scores are bounded by construction),
    denominators via ones-vector matmul, PV with v[s,d]-stationary
    accumulation -> outT [d, tq], normalization by broadcast reciprocal.
  - out projection: attn_flatT chunks stationary, Wo.T moving.
All matmuls run as float32r (full-rate fp32 PE mode).
"""

import numpy as np
from contextlib import ExitStack

import concourse.bass as bass
import concourse.bacc as bacc
import concourse.tile as tile
import concourse.mybir as mybir
from concourse.bass_utils import run_bass_kernel_spmd

# problem shape (hardcoded per contract)
B, T, E = 2, 2048, 2048
NH, NG, HD = 16, 4, 128
HPG = NH // NG          # 4 heads per group = per core
NE = E // 128           # 16 contraction chunks
TB = 512                # tq / t block
NTB = T // TB           # 4
NST = T // 128          # 16 s-tiles
F32 = mybir.dt.float32
F32R = mybir.dt.float32r
EXP = mybir.ActivationFunctionType.Exp

N_CORES = 8


def _r(ap):
    return ap.bitcast(F32R)


def build_body(tc, out_ap, ins):
    """ins: dict name -> dram AP. out_ap: [T, E] dram AP."""
    nc = tc.nc
    ctx = ExitStack()
    with ctx:
        ctx.enter_context(nc.allow_low_precision(
            reason="fp32r rounding on matmul inputs is intended"))
        # ---- constant / persistent SBUF ----
        const = ctx.enter_context(tc.tile_pool(name="const", bufs=1))
        cs2 = const.tile([128, T], F32, tag="cs2", name="cs2")
        snpm = const.tile([128, T], F32, tag="snpm", name="snpm")
        tri = const.tile([128, 128], F32, tag="tri", name="tri")
        swp = const.tile([128, 128], F32R, tag="swp", name="swp")
        iden = const.tile([128, 128], F32, tag="iden", name="iden")
        ones = const.tile([128, 128], F32R, tag="ones", name="ones")
        zer = const.tile([128, TB], F32, tag="zer", name="zer")
        nc.sync.dma_start(cs2[:], ins["cs2"][:])
        nc.sync.dma_start(snpm[:], ins["snpm"][:])
        nc.sync.dma_start(tri[:], ins["tri"][:])
        nc.sync.dma_start(swp[:], _r(ins["swp"][:]))
        nc.sync.dma_start(iden[:], ins["iden"][:])
        nc.sync.dma_start(ones[:], _r(ins["onec"][:]))
        nc.sync.dma_start(zer[:], ins["zer"][:])

        persist = ctx.enter_context(tc.tile_pool(name="persist", bufs=1))
        qrot = [persist.tile([128, T], F32, tag=f"qrot{h}", name=f"qrot{h}") for h in range(HPG)]
        krot = persist.tile([128, T], F32, tag="krot", name="krot")
        vsd = persist.tile([128, T], F32, tag="vsd", name="vsd")
        aout = qrot  # attn output overwrites qrot block-by-block (dead after scores)

        # ---- weights (packed into single wide tiles, col block = e-chunk) ----
        wpool = ctx.enter_context(tc.tile_pool(name="weights", bufs=1))
        wq_t = wpool.tile([128, NE * 512], F32R, tag="wbig", name="wq")    # block e: [128, 4*128]
        wk_t = wpool.tile([128, NE * 128], F32R, tag="wk", name="wk")
        wv_t = wpool.tile([128, NE * 128], F32R, tag="wv", name="wv")
        for e in range(NE):
            r0 = e * 128
            nc.sync.dma_start(wq_t[:, e * 512:(e + 1) * 512], _r(ins["wq"][r0:r0 + 128, :]))
            nc.sync.dma_start(wk_t[:, e * 128:(e + 1) * 128], _r(ins["wk"][r0:r0 + 128, :]))
            nc.sync.dma_start(wv_t[:, e * 128:(e + 1) * 128], _r(ins["wv"][r0:r0 + 128, :]))


        # ---- psum pools ----
        psp = ctx.enter_context(tc.tile_pool(name="psp", bufs=2, space="PSUM"))
        pswp = ctx.enter_context(tc.tile_pool(name="pswp", bufs=1, space="PSUM"))
        # attention-phase psum
        pssp = ctx.enter_context(tc.tile_pool(name="pssp", bufs=2, space="PSUM"))
        psop = ctx.enter_context(tc.tile_pool(name="psop", bufs=2, space="PSUM"))
        psdp = ctx.enter_context(tc.tile_pool(name="psdp", bufs=1, space="PSUM"))

        # ---- sbuf working pools ----
        xpool = ctx.enter_context(tc.tile_pool(name="xcol", bufs=2))
        qrpool = ctx.enter_context(tc.tile_pool(name="qraw", bufs=6))
        ptpool = ctx.enter_context(tc.tile_pool(name="pt", bufs=4))
        srpool = ctx.enter_context(tc.tile_pool(name="sr", bufs=4))
        ospool = ctx.enter_context(tc.tile_pool(name="osb", bufs=2))

        def rope(dst_ap, ps, cols):
            """dst = raw*cos + swap(raw)*sgn_sin, raw in psum ps [128, TB]."""
            qraw = qrpool.tile([128, TB], F32, tag="qraw", name="qraw")
            nc.scalar.copy(_r(qraw[:]), ps[:])
            ps_sw = pswp.tile([128, TB], F32, tag="psw", name="psw")
            nc.tensor.matmul(ps_sw[:], _r(swp[:]), _r(qraw[:]), start=True, stop=True)
            tmp1 = qrpool.tile([128, TB], F32, tag="qraw", name="ropetmp1")
            tmp2 = qrpool.tile([128, TB], F32, tag="qraw", name="ropetmp2")
            nc.vector.tensor_mul(tmp1[:], qraw[:], cs2[:, cols])
            nc.vector.tensor_mul(tmp2[:], ps_sw[:], snpm[:, cols])
            nc.vector.tensor_add(_r(dst_ap), tmp1[:], tmp2[:])

        # ================= projection phase =================
        for tb in range(NTB):
            cols = slice(tb * TB, (tb + 1) * TB)
            xc = []
            for h2 in range(2):
                t_ = xpool.tile([128, 8 * TB], F32R, tag="xc", name="xc")
                for i in range(8):
                    e = h2 * 8 + i
                    nc.sync.dma_start(
                        t_[:, i * TB:(i + 1) * TB],
                        _r(ins["xT"][e * 128:(e + 1) * 128, cols]))
                xc.append(t_)

            def xce(e):
                return xc[e // 8][:, (e % 8) * TB:(e % 8 + 1) * TB]

            for dq in range(HPG):
                ps = psp.tile([128, TB], F32, tag="ps", name="ps")
                for e in range(NE):
                    nc.tensor.matmul(
                        ps[:],
                        _r(wq_t[:, e * 512 + dq * 128: e * 512 + (dq + 1) * 128]),
                        _r(xce(e)), start=(e == 0), stop=(e == NE - 1))
                rope(qrot[dq][:, cols], ps, cols)

            ps_k = psp.tile([128, TB], F32, tag="ps", name="ps")
            for e in range(NE):
                nc.tensor.matmul(ps_k[:], _r(wk_t[:, e * 128:(e + 1) * 128]),
                                 _r(xce(e)), start=(e == 0), stop=(e == NE - 1))
            rope(krot[:, cols], ps_k, cols)

            ps_v = psp.tile([128, TB], F32, tag="ps", name="ps")
            for e in range(NE):
                nc.tensor.matmul(ps_v[:], _r(wv_t[:, e * 128:(e + 1) * 128]),
                                 _r(xce(e)), start=(e == 0), stop=(e == NE - 1))
            vtmp = qrpool.tile([128, TB], F32, tag="qraw", name="vtmp")
            nc.scalar.copy(vtmp[:], ps_v[:])

            # transpose v tiles of this block: vtmp [d, s] -> vsd [s, d]
            for jj in range(4):
                j = 4 * tb + jj
                pst = psp.tile([128, 128], F32, tag="ps", name="ps")
                nc.tensor.transpose(pst[:], vtmp[:, jj * 128:(jj + 1) * 128], iden[:])
                nc.scalar.copy(_r(vsd[:, j * 128:(j + 1) * 128]), pst[:])

        # wo: packed [128, 16*512], col block (hh*4+eo)
        wo_t = wpool.tile([128, NE * 512], F32R, tag="wbig", name="wo")
        for hh in range(HPG):
            for eo in range(4):
                blk = hh * 4 + eo
                nc.sync.dma_start(
                    wo_t[:, blk * 512:(blk + 1) * 512],
                    _r(ins["wo"][hh * 128:(hh + 1) * 128, eo * 512:(eo + 1) * 512]))

        # ================= attention phase =================
        for bi in range(NTB):
            for h in range(HPG):
                jmax = 4 * bi + 3
                pso = psop.tile([128, TB], F32, tag="pso", name="pso")
                psd = psdp.tile([128, TB], F32, tag="psd", name="psd")
                for j in range(jmax + 1):
                    diag = (j // 4 == bi)
                    o = 128 * (j - 4 * bi) if diag else 0
                    oe = min(o, 256)
                    W = TB - oe
                    pss = pssp.tile([128, TB], F32, tag="pss", name="pss")
                    nc.tensor.matmul(
                        pss[:, 0:W],
                        _r(krot[:, j * 128:(j + 1) * 128]),
                        _r(qrot[h][:, bi * TB + oe:(bi + 1) * TB]),
                        start=True, stop=True)
                    pt = ptpool.tile([128, TB], F32, tag="pt", name="pt")
                    nc.scalar.activation(_r(pt[:, oe:TB]), pss[:, 0:W], EXP)
                    if diag:
                        if o > 0:
                            nc.vector.tensor_copy(_r(pt[:, 0:o]), zer[:, 0:o])
                        nc.vector.tensor_mul(_r(pt[:, o:o + 128]), pt[:, o:o + 128], tri[:])
                    nc.tensor.matmul(psd[:], _r(ones[:]), _r(pt[:]),
                                     start=(j == 0), stop=(j == jmax))
                    nc.tensor.matmul(pso[:], _r(vsd[:, j * 128:(j + 1) * 128]),
                                     _r(pt[:]), start=(j == 0), stop=(j == jmax))
                cols = slice(bi * TB, (bi + 1) * TB)
                rden = srpool.tile([128, TB], F32, tag="rden", name="rden")
                nc.vector.reciprocal(rden[:], psd[:])
                nc.vector.tensor_mul(_r(aout[h][:, cols]), pso[:], rden[:])

        # ================= output projection =================
        for tq in range(NST):
            trows = slice(tq * 128, (tq + 1) * 128)
            for half in range(2):
                poa = pssp.tile([128, TB], F32, tag="pss", name="pss")
                pob = psop.tile([128, TB], F32, tag="pso", name="pso")
                for hh in range(HPG):
                    lh = _r(aout[hh][:, trows])
                    ba = hh * 4 + 2 * half
                    nc.tensor.matmul(poa[:], lh, _r(wo_t[:, ba * 512:(ba + 1) * 512]),
                                     start=(hh == 0), stop=(hh == HPG - 1))
                    nc.tensor.matmul(pob[:], lh, _r(wo_t[:, (ba + 1) * 512:(ba + 2) * 512]),
                                     start=(hh == 0), stop=(hh == HPG - 1))
                for k, po in ((0, poa), (1, pob)):
                    eo = 2 * half + k
                    osb = ospool.tile([128, TB], F32, tag="osb", name="osb")
                    nc.scalar.copy(osb[:], po[:])
                    nc.sync.dma_start(out_ap[trows, eo * 512:(eo + 1) * 512], osb[:])


# ---------------- host side ----------------

_PERM = np.concatenate([np.arange(0, HD, 2), np.arange(1, HD, 2)])  # half-split


def host_prep(inputs):
    """Full inputs -> list of 8 per-core input dicts (core i = (b=i//4, g=i%4))."""
    x = np.asarray(inputs["x"], dtype=np.float32)
    Wq = np.asarray(inputs["Wq"], dtype=np.float32)
    Wk = np.asarray(inputs["Wk"], dtype=np.float32)
    Wv = np.asarray(inputs["Wv"], dtype=np.float32)
    Wo = np.asarray(inputs["Wo"], dtype=np.float32)

    inv = (10000.0 ** (-np.arange(0, HD, 2, dtype=np.float32) / HD)).astype(np.float32)
    tpos = np.arange(T, dtype=np.float32)
    fr = np.outer(tpos, inv)                       # [T, 64]
    cosT = np.cos(fr).T.astype(np.float32)         # [64, T]
    sinT = np.sin(fr).T.astype(np.float32)
    cs2 = np.concatenate([cosT, cosT], axis=0)     # [128, T]
    snpm = np.concatenate([-sinT, sinT], axis=0)   # [128, T]

    tri = (np.arange(128)[None, :] >= np.arange(128)[:, None]).astype(np.float32)
    swp = np.zeros((128, 128), dtype=np.float32)
    swp[(np.arange(128) + 64) % 128, np.arange(128)] = 1.0
    iden = np.eye(128, dtype=np.float32)

    scale = np.float32(1.0 / np.sqrt(HD))
    xT = [np.ascontiguousarray(x[b].T) for b in range(B)]

    in_maps = []
    for i in range(N_CORES):
        b, g = i // 4, i % 4
        # wq: rows for heads g*4..g*4+3, each permuted, scaled; -> [E, 512]
        rows = []
        for h in range(HPG):
            base = (g * HPG + h) * HD
            rows.append(Wq[base + _PERM, :])
        wq_c = (np.concatenate(rows, axis=0) * scale).T  # [E, 512]
        wk_c = Wk[g * HD + _PERM, :].T                   # [E, 128]
        wv_c = Wv[g * HD:(g + 1) * HD, :].T              # [E, 128]
        wo_c = np.ascontiguousarray(Wo[:, g * 512:(g + 1) * 512].T)  # [512, E]
        in_maps.append({
            "xT": xT[b],
            "wq": np.ascontiguousarray(wq_c),
            "wk": np.ascontiguousarray(wk_c),
            "wv": np.ascontiguousarray(wv_c),
            "wo": wo_c,
            "cs2": cs2, "snpm": snpm, "tri": tri, "swp": swp, "iden": iden,
            "onec": np.ones((128, 128), dtype=np.float32),
            "zer": np.zeros((128, TB), dtype=np.float32),
        })
    return in_maps


_NC = None


def build_nc():
    global _NC
    if _NC is not None:
        return _NC
    nc = bacc.Bacc("TRN2", target_bir_lowering=False, debug=False,
                   num_devices=N_CORES)
    ins = {
        "xT": nc.dram_tensor("xT", [E, T], F32R, kind="ExternalInput").ap(),
        "wq": nc.dram_tensor("wq", [E, HPG * HD], F32R, kind="ExternalInput").ap(),
        "wk": nc.dram_tensor("wk", [E, HD], F32R, kind="ExternalInput").ap(),
        "wv": nc.dram_tensor("wv", [E, HD], F32R, kind="ExternalInput").ap(),
        "wo": nc.dram_tensor("wo", [HPG * HD, E], F32R, kind="ExternalInput").ap(),
        "cs2": nc.dram_tensor("cs2", [128, T], F32, kind="ExternalInput").ap(),
        "snpm": nc.dram_tensor("snpm", [128, T], F32, kind="ExternalInput").ap(),
        "tri": nc.dram_tensor("tri", [128, 128], F32, kind="ExternalInput").ap(),
        "swp": nc.dram_tensor("swp", [128, 128], F32R, kind="ExternalInput").ap(),
        "iden": nc.dram_tensor("iden", [128, 128], F32, kind="ExternalInput").ap(),
        "onec": nc.dram_tensor("onec", [128, 128], F32R, kind="ExternalInput").ap(),
        "zer": nc.dram_tensor("zer", [128, TB], F32, kind="ExternalInput").ap(),
    }
    out = nc.dram_tensor("out", [T, E], F32, kind="ExternalOutput").ap()
    with tile.TileContext(nc) as tc:
        build_body(tc, out, ins)
    nc.compile()
    _NC = nc
    return nc


def gather(results):
    """results: list of 8 dicts with 'out' [T, E] partials -> [B, T, E]."""
    out = np.zeros((B, T, E), dtype=np.float32)
    for i in range(N_CORES):
        out[i // 4] += results[i]["out"]
    return out


def kernel(**inputs):
    nc = build_nc()
    in_maps = host_prep(inputs)
    res = run_bass_kernel_spmd(nc, in_maps, core_ids=list(range(N_CORES)))
    return gather(res.results)


if __name__ == "__main__":
    rng = np.random.default_rng(0)
    ins = {
        "x": rng.standard_normal((B, T, E), dtype=np.float32),
        "Wq": rng.standard_normal((E, E), dtype=np.float32) * 0.02,
        "Wk": rng.standard_normal((NG * HD, E), dtype=np.float32) * 0.02,
        "Wv": rng.standard_normal((NG * HD, E), dtype=np.float32) * 0.02,
        "Wo": rng.standard_normal((E, E), dtype=np.float32) * 0.02,
    }
    out = kernel(**ins)
    print(out.shape, out.dtype, np.abs(out).mean())
